# revision 45
# baseline (speedup 1.0000x reference)
import sys

sys.path.insert(0, "/opt/trn_rl_repo")
import numpy as np
import ml_dtypes
import concourse.bass as bass
import concourse.bacc as bacc
import concourse.mybir as mybir
import concourse.tile as tile
from concourse.bass_utils import run_bass_kernel_spmd

F32R = mybir.dt.float32r
F32 = mybir.dt.float32
BF16 = mybir.dt.bfloat16
F16 = mybir.dt.float16
AF = mybir.ActivationFunctionType

B, S, D, H, DV = 2, 2048, 1024, 16, 64
NKT = 8     # k-tiles of 128 over D
NJ = 4      # query chunks of 512
NB = 16     # key blocks of 128
HPC = 4     # heads per core
DOFF = [0, 512, 1024, 1280]  # diag-pack column offsets (bank-aligned: dd2/dd3 share bank 2)
DW = [512, 384, 256, 128]    # diag-pack widths

_NC = None


def _build(debug=False):
    nc = bacc.Bacc(target_bir_lowering=False)
    xq = nc.dram_tensor("xq", [D, S], F16, kind="ExternalInput")
    xk = nc.dram_tensor("xk", [D, S], F16, kind="ExternalInput")
    xv = nc.dram_tensor("xv", [D, S], BF16, kind="ExternalInput")
    wq = nc.dram_tensor("wq", [D, 256], F16, kind="ExternalInput")
    wk = nc.dram_tensor("wk", [D, 256], F16, kind="ExternalInput")
    wv = nc.dram_tensor("wv", [D, 256], BF16, kind="ExternalInput")
    w0 = nc.dram_tensor("w0", [256, D], F16, kind="ExternalInput")
    cm = nc.dram_tensor("cm", [4, 128, 512], BF16, kind="ExternalInput")
    yt = nc.dram_tensor("yt", [D, S], F16, kind="ExternalOutput")

    with tile.TileContext(nc) as tc:
        with tc.tile_pool(name="pp", bufs=1) as pp:
            # per-(pair, jj) tiles so attention deps are chunk-granular
            qt_sb = [[pp.tile([128, 512], F16, name=f"qtsb{i}{jj}", tag=f"qtsb{i}{jj}")
                      for jj in range(4)] for i in range(2)]
            kt_sb = [[pp.tile([128, 512], F16, name=f"ktsb{i}{jj}", tag=f"ktsb{i}{jj}")
                      for jj in range(4)] for i in range(2)]
            v_sb = pp.tile([128, NB, HPC, 65], BF16, name="vsb", tag="vsb")
            w0_sb = [pp.tile([128, D], F16, name=f"w0sb{p}", tag=f"w0sb{p}") for p in range(2)]
            ot_sb = [pp.tile([128, S], F16, name=f"otsb{p}", tag=f"otsb{p}") for p in range(2)]
            cm_sb = pp.tile([128, 4, 512], BF16, name="cmsb", tag="cmsb")
            ones65 = pp.tile([65, 64], F32R, name="ones65", tag="ones65")
            onestage = pp.tile([65, 64], F32, name="onestage", tag="onestage")
            vstage = pp.tile([128, NB, HPC], BF16, name="vstage", tag="vstage")

            # each weight tensor in ONE combined DMA, placed at the head of a
            # fast queue ahead of x so the projections are never weight-gated
            def wload(w, dt, nm, eng):
                t = pp.tile([128, NKT, 256], dt, name=nm, tag=nm)
                eng.dma_start(out=t[:, :, :],
                              in_=w[:, :].rearrange("(f p) s -> p f s", f=NKT))
                return lambda kt: t[:, kt, :]
            wv_t = wload(wv, BF16, "wv", nc.sync)
            wq_t = wload(wq, F16, "wq", nc.scalar)
            wk_t = wload(wk, F16, "wk", nc.gpsimd)
            nc.gpsimd.dma_start(out=cm_sb[:, :, :],
                                in_=cm[:, :, :].rearrange("f p s -> p f s"))
            for p in range(2):
                nc.gpsimd.dma_start(out=w0_sb[p][:, :], in_=w0[128 * p:128 * p + 128, :])
            nc.vector.memset(onestage[64:65, :], 1.0)
            nc.vector.tensor_copy(ones65[64:65, :], onestage[64:65, :])
            nc.vector.memset(vstage[:, :, :], 1.0)
            nc.vector.tensor_copy(v_sb[:, :, :, 64], vstage[:, :, :])
            # preload the exp table set during phase A (ACT is idle there)
            expwarm = pp.tile([1, 64], F32, name="expwarm", tag="expwarm")
            nc.scalar.activation(expwarm[0:1, :], onestage[64:65, :], AF.Exp)

            # ---- Phase A: projections (kt-outer, xv -> xq -> xk) ----
            with tc.tile_pool(name="xin", bufs=1) as xin, \
                 tc.tile_pool(name="psA", bufs=8, space="PSUM") as psA:
                # x inputs: two kt chunks per DMA (1MB transfers), interleaved
                # across the sync and scalar queues, arrival order xv->xq->xk
                flip = [True]

                def xload(x, dt, nm, tag, bufs):
                    tiles = []
                    for c in range(NKT // 2):
                        t = xin.tile([128, 2, S], dt, name=f"{nm}{c}", tag=tag,
                                     bufs=bufs)
                        eng = nc.sync if flip[0] else nc.scalar
                        flip[0] = not flip[0]
                        eng.dma_start(
                            out=t[:, :, :],
                            in_=x[256 * c:256 * c + 256, :].rearrange(
                                "(two p) s -> p two s", two=2))
                        tiles.append(t)
                    return lambda kt: tiles[kt // 2][:, kt % 2, :]
                xv_t = xload(xv, BF16, "xv", "xv", 4)
                xq_t = xload(xq, F16, "xq", "x", 8)
                xk_t = xload(xk, F16, "xk", "x", 8)

                # V projection: 2 waves x 8 st-groups, kt-outer within a wave
                for w in range(2):
                    vps = [psA.tile([128, HPC, 64], F32, name=f"vps{w}{g}", tag="pj")
                           for g in range(8)]
                    for kt in range(NKT):
                        for g in range(8):
                            st = 8 * w + g
                            nc.tensor.matmul(
                                vps[g][:, :, :],
                                xv_t(kt)[:, 128 * st:128 * st + 128],
                                wv_t(kt)[:, :],
                                start=(kt == 0), stop=(kt == NKT - 1))
                    for g in range(8):
                        if g % 2:
                            nc.scalar.copy(v_sb[:, 8 * w + g, :, 0:64], vps[g][:, :, :])
                        else:
                            nc.vector.tensor_copy(v_sb[:, 8 * w + g, :, 0:64],
                                                  vps[g][:, :, :])

                # QT / KT: kt-outer, all 8 (p, jj) psum groups live
                for which, wt, xt, dst in (("q", wq_t, xq_t, qt_sb), ("k", wk_t, xk_t, kt_sb)):
                    qps = [psA.tile([128, 512], F32, name=f"{which}ps{i}", tag="pj")
                           for i in range(8)]
                    for kt in range(NKT):
                        for p in range(2):
                            for jj in range(4):
                                nc.tensor.matmul(
                                    qps[4 * p + jj][:, :],
                                    wt(kt)[:, 128 * p:128 * p + 128],
                                    xt(kt)[:, 512 * jj:512 * jj + 512],
                                    start=(kt == 0), stop=(kt == NKT - 1))
                    for jj in range(4):
                        for p in range(2):
                            # split copies across DVE and ACT so the pool-exit
                            # barrier after the last projection drains 2x faster
                            if p:
                                nc.scalar.copy(dst[p][jj][:, :], qps[4 * p + jj][:, :])
                            else:
                                nc.vector.tensor_copy(dst[p][jj][:, :],
                                                      qps[4 * p + jj][:, :])

            # ---- Phase B/C interleaved: attention (j-outer) + out-proj ----
            with tc.tile_pool(name="pb", bufs=1) as pb, \
                 tc.tile_pool(name="psB", bufs=1, space="PSUM") as psB:

                def emit_norm(h, j, opsum):
                    # copy the whole opsum (numerators + den row) to SBUF in
                    # one op so the PSUM slot frees immediately; the rest of
                    # the norm chain runs from the copy.
                    den = pb.tile([65, 512], F32R, name="den", tag="den", bufs=2)
                    nc.vector.tensor_copy(den[:, :], opsum[0:65, :])
                    bcps = psB.tile([64, 512], F32, name="bcps", tag="acc", bufs=2)
                    nc.tensor.matmul(bcps[:, :], ones65[64:65, :], den[64:65, :],
                                     start=True, stop=True)
                    rec = pb.tile([64, 512], F32, name="rec", tag="rec", bufs=2)
                    nc.vector.reciprocal_approx_fast(rec[:, :], bcps[:, :])
                    nc.vector.tensor_mul(
                        ot_sb[h // 2][64 * (h % 2):64 * (h % 2) + 64,
                                      512 * j:512 * j + 512],
                        den[0:64, :], rec[:, :])

                def emit_phase_c(j, es, alt=False):
                    for e in es:
                        yps = psB.tile([128, 512], F32, name="yps", tag="acc", bufs=2)
                        for p in range(2):
                            nc.tensor.matmul(
                                yps[:, :],
                                w0_sb[p][:, 128 * e:128 * e + 128],
                                ot_sb[p][:, 512 * j:512 * j + 512],
                                start=(p == 0), stop=(p == 1))
                        ysb = pb.tile([128, 512], F16, name="ysb", tag="ysb", bufs=4)
                        if alt and e % 2:
                            # tail: ACT is idle, split the casts across engines
                            nc.scalar.copy(ysb[:, :], yps[:, :])
                        else:
                            nc.vector.tensor_copy(ysb[:, :], yps[:, :])
                        nc.sync.dma_start(out=yt[128 * e:128 * e + 128, 512 * j:512 * j + 512],
                                          in_=ysb[:, :])

                # Flatten all (unit, trip) pairs into one global stream so the
                # score matmuls stay 2 trips ahead of exp ACROSS unit
                # boundaries — ACT (the phase-B bottleneck) never starves.
                JORD = [0, 1, 2, 3]
                steps = []  # (j, h, trips, t, prev_j)
                for bi, j in enumerate(JORD):
                    for h in range(HPC):
                        offs = list(range(4 * j))
                        trips = [offs[t:t + 3] for t in range(0, len(offs), 3)] + ["diag"]
                        for t in range(len(trips)):
                            steps.append((j, h, trips, t,
                                          JORD[bi - 1] if bi > 0 else None))
                M = len(steps)
                st_tiles = {}
                opsums = {}

                def emit_scores(m):
                    j, h, trips, t, _ = steps[m]
                    pair, pbase = h // 2, 64 * (h % 2)
                    stile = psB.tile([128, 1536], F32, name="stile", tag="stile", bufs=2)
                    st_tiles[m] = stile
                    if trips[t] == "diag":
                        for dd in range(4):
                            i = 4 * j + dd
                            nc.tensor.matmul(
                                stile[:, DOFF[dd]:DOFF[dd] + DW[dd]],
                                kt_sb[pair][i // 4][pbase:pbase + 64,
                                                    128 * (i % 4):128 * (i % 4) + 128],
                                qt_sb[pair][j][pbase:pbase + 64, 128 * dd:512],
                                start=(dd != 3), stop=(dd != 2))
                    else:
                        for n, i in enumerate(trips[t]):
                            nc.tensor.matmul(
                                stile[:, 512 * n:512 * n + 512],
                                kt_sb[pair][i // 4][pbase:pbase + 64,
                                                    128 * (i % 4):128 * (i % 4) + 128],
                                qt_sb[pair][j][pbase:pbase + 64, :],
                                start=True, stop=True)

                emit_scores(0)
                emit_scores(1)
                for m in range(M):
                    j, h, trips, t, prev_j = steps[m]
                    ntrip = len(trips)
                    if t == 0:
                        if prev_j is not None:
                            emit_phase_c(prev_j, [2 * h, 2 * h + 1])
                        opsums[(j, h)] = psB.tile([128, 512], F32, name="opsum",
                                                  tag="acc", bufs=2)
                    opsum = opsums[(j, h)]
                    ptt = pb.tile([128, 1536], BF16, name="ptt", tag="ptt", bufs=4)
                    if trips[t] == "diag":
                        # one wide exp covering the 896:1024 gap (cols there
                        # are stale psum, never read by PV)
                        nc.scalar.activation(ptt[:, 0:1408], st_tiles[m][:, 0:1408],
                                             AF.Exp)
                        for dd in range(4):
                            nc.gpsimd.tensor_mul(
                                ptt[:, DOFF[dd]:DOFF[dd] + 128],
                                ptt[:, DOFF[dd]:DOFF[dd] + 128],
                                cm_sb[:, dd, 128 * dd:128 * dd + 128])
                    else:
                        width = 512 * len(trips[t])
                        nc.scalar.activation(ptt[:, 0:width], st_tiles[m][:, 0:width],
                                             AF.Exp)
                    if m + 2 < M:
                        emit_scores(m + 2)
                    if trips[t] == "diag":
                        for dd in range(4):
                            nc.tensor.matmul(
                                opsum[0:65, 128 * dd:512],
                                v_sb[:, 4 * j + dd, h, :],
                                ptt[:, DOFF[dd]:DOFF[dd] + DW[dd]],
                                start=(j == 0 and dd == 0), stop=(dd == 3))
                    else:
                        for n, i in enumerate(trips[t]):
                            nc.tensor.matmul(
                                opsum[0:65, :],
                                v_sb[:, i, h, :],
                                ptt[:, 512 * n:512 * n + 512],
                                start=(t == 0 and n == 0), stop=False)
                    if t == ntrip - 1:
                        emit_norm(h, j, opsum)
                        del opsums[(j, h)]
                emit_phase_c(JORD[-1], list(range(8)), alt=True)

    nc.compile()
    return nc


def _run(inputs, trace=False, debug=False, tmpdir=None):
    global _NC
    if _NC is None:
        _NC = _build(debug=debug)
    q = np.asarray(inputs["q"], dtype=np.float32)
    k = np.asarray(inputs["k"], dtype=np.float32)
    v = np.asarray(inputs["v"], dtype=np.float32)
    mask = np.asarray(inputs["mask"])
    w_query = np.asarray(inputs["w_query"], dtype=np.float32)
    w_key = np.asarray(inputs["w_key"], dtype=np.float32)
    w_value = np.asarray(inputs["w_value"], dtype=np.float32)
    w_0 = np.asarray(inputs["w_0"], dtype=np.float32)

    cmask = np.stack([
        np.ascontiguousarray(mask[0, 0, 0:512, 128 * i:128 * i + 128].T)
        for i in range(4)
    ]).astype(ml_dtypes.bfloat16)
    xq_b = [np.ascontiguousarray(q[b].T).astype(np.float16) for b in range(B)]
    xk_b = [np.ascontiguousarray(k[b].T).astype(np.float16) for b in range(B)]
    xv_b = [np.ascontiguousarray(v[b].T).astype(ml_dtypes.bfloat16) for b in range(B)]

    in_maps = []
    for c in range(8):
        b, g = c // 4, c % 4
        sl = slice(256 * g, 256 * g + 256)
        in_maps.append({
            "xq": xq_b[b], "xk": xk_b[b], "xv": xv_b[b],
            "wq": np.ascontiguousarray(w_query[sl, :].T).astype(np.float16),
            "wk": np.ascontiguousarray(w_key[sl, :].T).astype(np.float16),
            "wv": np.ascontiguousarray(w_value[sl, :].T).astype(ml_dtypes.bfloat16),
            "w0": np.ascontiguousarray(w_0[:, sl].T).astype(np.float16),
            "cm": cmask,
        })

    res = run_bass_kernel_spmd(_NC, in_maps, core_ids=list(range(8)), trace=trace,
                               tmpdir=tmpdir)
    y = np.empty((B, S, D), dtype=np.float32)
    for b in range(B):
        acc = res.results[4 * b]["yt"].astype(np.float32)
        for g in range(1, 4):
            acc += res.results[4 * b + g]["yt"].astype(np.float32)
        y[b] = acc.T
    if debug:
        return y, getattr(res, "exec_time_ns", None), res
    return y, getattr(res, "exec_time_ns", None)


def kernel(**inputs):
    return _run(inputs, trace=False)[0]


# revision 48
# speedup vs baseline: 1.0237x; 1.0237x over previous
import sys

sys.path.insert(0, "/opt/trn_rl_repo")
import numpy as np
import ml_dtypes
import concourse.bass as bass
import concourse.bacc as bacc
import concourse.mybir as mybir
import concourse.tile as tile
from concourse.bass_utils import run_bass_kernel_spmd

F32R = mybir.dt.float32r
F32 = mybir.dt.float32
BF16 = mybir.dt.bfloat16
F16 = mybir.dt.float16
AF = mybir.ActivationFunctionType

B, S, D, H, DV = 2, 2048, 1024, 16, 64
NKT = 8     # k-tiles of 128 over D
NJ = 4      # query chunks of 512
NB = 16     # key blocks of 128
HPC = 4     # heads per core
DOFF = [0, 512, 1024, 1280]  # diag-pack column offsets (bank-aligned: dd2/dd3 share bank 2)
DW = [512, 384, 256, 128]    # diag-pack widths

_NC = None


def _build(debug=False):
    nc = bacc.Bacc(target_bir_lowering=False)
    xq = nc.dram_tensor("xq", [D, S], F16, kind="ExternalInput")
    xk = nc.dram_tensor("xk", [D, S], F16, kind="ExternalInput")
    xv = nc.dram_tensor("xv", [D, S], BF16, kind="ExternalInput")
    wq = nc.dram_tensor("wq", [D, 256], F16, kind="ExternalInput")
    wk = nc.dram_tensor("wk", [D, 256], F16, kind="ExternalInput")
    wv = nc.dram_tensor("wv", [D, 256], BF16, kind="ExternalInput")
    w0 = nc.dram_tensor("w0", [256, D], F16, kind="ExternalInput")
    cm = nc.dram_tensor("cm", [4, 128, 512], BF16, kind="ExternalInput")
    yt = nc.dram_tensor("yt", [D, S], F16, kind="ExternalOutput")

    with tile.TileContext(nc) as tc:
        with tc.tile_pool(name="pp", bufs=1) as pp:
            # per-(pair, jj) tiles so attention deps are chunk-granular
            qt_sb = [[pp.tile([128, 512], F16, name=f"qtsb{i}{jj}", tag=f"qtsb{i}{jj}")
                      for jj in range(4)] for i in range(2)]
            kt_sb = [[pp.tile([128, 512], F16, name=f"ktsb{i}{jj}", tag=f"ktsb{i}{jj}")
                      for jj in range(4)] for i in range(2)]
            v_sb = pp.tile([128, NB, HPC, 65], BF16, name="vsb", tag="vsb")
            w0_sb = [pp.tile([128, D], F16, name=f"w0sb{p}", tag=f"w0sb{p}") for p in range(2)]
            ot_sb = [pp.tile([128, S], F16, name=f"otsb{p}", tag=f"otsb{p}") for p in range(2)]
            cm_sb = pp.tile([128, 4, 512], BF16, name="cmsb", tag="cmsb")
            ones65 = pp.tile([65, 64], F32R, name="ones65", tag="ones65")
            onestage = pp.tile([65, 64], F32, name="onestage", tag="onestage")
            vstage = pp.tile([128, NB, HPC], BF16, name="vstage", tag="vstage")

            # weights first (V proj starts earliest), then cm/w0 (phase B/C).
            # two 128-row kt chunks per DMA (bigger transfers run faster).
            def wload(w, dt, nm):
                tiles = []
                for c in range(NKT // 2):
                    t = pp.tile([128, 2, 256], dt, name=f"{nm}{c}", tag=f"{nm}{c}")
                    nc.gpsimd.dma_start(
                        out=t[:, :, :],
                        in_=w[256 * c:256 * c + 256, :].rearrange(
                            "(two p) s -> p two s", two=2))
                    tiles.append(t)
                return lambda kt: tiles[kt // 2][:, kt % 2, :]
            wv_t = wload(wv, BF16, "wv")
            wq_t = wload(wq, F16, "wq")
            wk_t = wload(wk, F16, "wk")
            nc.gpsimd.dma_start(out=cm_sb[:, :, :],
                                in_=cm[:, :, :].rearrange("f p s -> p f s"))
            for p in range(2):
                nc.gpsimd.dma_start(out=w0_sb[p][:, :], in_=w0[128 * p:128 * p + 128, :])
            nc.vector.memset(onestage[64:65, :], 1.0)
            nc.vector.tensor_copy(ones65[64:65, :], onestage[64:65, :])
            nc.vector.memset(vstage[:, :, :], 1.0)
            nc.vector.tensor_copy(v_sb[:, :, :, 64], vstage[:, :, :])
            # preload the exp table set during phase A (ACT is idle there)
            expwarm = pp.tile([1, 64], F32, name="expwarm", tag="expwarm")
            nc.scalar.activation(expwarm[0:1, :], onestage[64:65, :], AF.Exp)

            # ---- Phase A: projections (kt-outer, xv -> xq -> xk) ----
            with tc.tile_pool(name="xin", bufs=1) as xin, \
                 tc.tile_pool(name="psA", bufs=8, space="PSUM") as psA:
                # x inputs: two kt chunks per DMA (1MB transfers), interleaved
                # across the sync and scalar queues, arrival order xv->xq->xk
                flip = [True]

                def xload(x, dt, nm, tag, bufs):
                    tiles = []
                    for c in range(NKT // 2):
                        t = xin.tile([128, 2, S], dt, name=f"{nm}{c}", tag=tag,
                                     bufs=bufs)
                        eng = nc.sync if flip[0] else nc.scalar
                        flip[0] = not flip[0]
                        eng.dma_start(
                            out=t[:, :, :],
                            in_=x[256 * c:256 * c + 256, :].rearrange(
                                "(two p) s -> p two s", two=2))
                        tiles.append(t)
                    return lambda kt: tiles[kt // 2][:, kt % 2, :]
                xv_t = xload(xv, BF16, "xv", "xv", 4)
                xq_t = xload(xq, F16, "xq", "x", 8)
                xk_t = xload(xk, F16, "xk", "x", 8)

                # V projection: 2 waves x 8 st-groups, kt-outer within a wave
                for w in range(2):
                    vps = [psA.tile([128, HPC, 64], F32, name=f"vps{w}{g}", tag="pj")
                           for g in range(8)]
                    for kt in range(NKT):
                        for g in range(8):
                            st = 8 * w + g
                            nc.tensor.matmul(
                                vps[g][:, :, :],
                                xv_t(kt)[:, 128 * st:128 * st + 128],
                                wv_t(kt)[:, :],
                                start=(kt == 0), stop=(kt == NKT - 1))
                    for g in range(8):
                        if g % 2:
                            nc.scalar.copy(v_sb[:, 8 * w + g, :, 0:64], vps[g][:, :, :])
                        else:
                            nc.vector.tensor_copy(v_sb[:, 8 * w + g, :, 0:64],
                                                  vps[g][:, :, :])

                # QT / KT: kt-outer, all 8 (p, jj) psum groups live
                for which, wt, xt, dst in (("q", wq_t, xq_t, qt_sb), ("k", wk_t, xk_t, kt_sb)):
                    qps = [psA.tile([128, 512], F32, name=f"{which}ps{i}", tag="pj")
                           for i in range(8)]
                    for kt in range(NKT):
                        for p in range(2):
                            for jj in range(4):
                                nc.tensor.matmul(
                                    qps[4 * p + jj][:, :],
                                    wt(kt)[:, 128 * p:128 * p + 128],
                                    xt(kt)[:, 512 * jj:512 * jj + 512],
                                    start=(kt == 0), stop=(kt == NKT - 1))
                    for jj in range(4):
                        for p in range(2):
                            # split copies across DVE and ACT so the pool-exit
                            # barrier after the last projection drains 2x faster
                            if p:
                                nc.scalar.copy(dst[p][jj][:, :], qps[4 * p + jj][:, :])
                            else:
                                nc.vector.tensor_copy(dst[p][jj][:, :],
                                                      qps[4 * p + jj][:, :])

            # ---- Phase B/C interleaved: attention (j-outer) + out-proj ----
            with tc.tile_pool(name="pb", bufs=1) as pb, \
                 tc.tile_pool(name="psB", bufs=1, space="PSUM") as psB:

                def emit_norm(h, j, opsum):
                    # copy the whole opsum (numerators + den row) to SBUF in
                    # one op so the PSUM slot frees immediately; the rest of
                    # the norm chain runs from the copy.
                    den = pb.tile([65, 512], F32R, name="den", tag="den", bufs=3)
                    nc.vector.tensor_copy(den[:, :], opsum[0:65, :])
                    bcps = psB.tile([64, 512], F32, name="bcps", tag="acc", bufs=2)
                    nc.tensor.matmul(bcps[:, :], ones65[64:65, :], den[64:65, :],
                                     start=True, stop=True)
                    rec = pb.tile([64, 512], F32, name="rec", tag="rec", bufs=3)
                    nc.vector.reciprocal_approx_fast(rec[:, :], bcps[:, :])
                    nc.vector.tensor_mul(
                        ot_sb[h // 2][64 * (h % 2):64 * (h % 2) + 64,
                                      512 * j:512 * j + 512],
                        den[0:64, :], rec[:, :])

                def emit_phase_c(j, es, alt=False):
                    for e in es:
                        yps = psB.tile([128, 512], F32, name="yps", tag="acc", bufs=2)
                        for p in range(2):
                            nc.tensor.matmul(
                                yps[:, :],
                                w0_sb[p][:, 128 * e:128 * e + 128],
                                ot_sb[p][:, 512 * j:512 * j + 512],
                                start=(p == 0), stop=(p == 1))
                        ysb = pb.tile([128, 512], F16, name="ysb", tag="ysb", bufs=4)
                        if alt and e % 2:
                            # tail: ACT is idle, split the casts across engines
                            nc.scalar.copy(ysb[:, :], yps[:, :])
                        else:
                            nc.vector.tensor_copy(ysb[:, :], yps[:, :])
                        nc.sync.dma_start(out=yt[128 * e:128 * e + 128, 512 * j:512 * j + 512],
                                          in_=ysb[:, :])

                # Flatten all (unit, trip) pairs into one global stream so the
                # score matmuls stay 2 trips ahead of exp ACROSS unit
                # boundaries — ACT (the phase-B bottleneck) never starves.
                JORD = [0, 1, 2, 3]
                steps = []  # (j, h, trips, t, prev_j)
                for bi, j in enumerate(JORD):
                    for h in range(HPC):
                        offs = list(range(4 * j))
                        trips = [offs[t:t + 3] for t in range(0, len(offs), 3)] + ["diag"]
                        for t in range(len(trips)):
                            steps.append((j, h, trips, t,
                                          JORD[bi - 1] if bi > 0 else None))
                M = len(steps)
                st_tiles = {}
                opsums = {}

                def emit_scores(m):
                    j, h, trips, t, _ = steps[m]
                    pair, pbase = h // 2, 64 * (h % 2)
                    stile = psB.tile([128, 1536], F32, name="stile", tag="stile", bufs=2)
                    st_tiles[m] = stile
                    if trips[t] == "diag":
                        for dd in range(4):
                            i = 4 * j + dd
                            nc.tensor.matmul(
                                stile[:, DOFF[dd]:DOFF[dd] + DW[dd]],
                                kt_sb[pair][i // 4][pbase:pbase + 64,
                                                    128 * (i % 4):128 * (i % 4) + 128],
                                qt_sb[pair][j][pbase:pbase + 64, 128 * dd:512],
                                start=(dd != 3), stop=(dd != 2))
                    else:
                        for n, i in enumerate(trips[t]):
                            nc.tensor.matmul(
                                stile[:, 512 * n:512 * n + 512],
                                kt_sb[pair][i // 4][pbase:pbase + 64,
                                                    128 * (i % 4):128 * (i % 4) + 128],
                                qt_sb[pair][j][pbase:pbase + 64, :],
                                start=True, stop=True)

                emit_scores(0)
                emit_scores(1)
                for m in range(M):
                    j, h, trips, t, prev_j = steps[m]
                    ntrip = len(trips)
                    if t == 0:
                        if prev_j is not None:
                            emit_phase_c(prev_j, [2 * h, 2 * h + 1])
                        opsums[(j, h)] = psB.tile([128, 512], F32, name="opsum",
                                                  tag="acc", bufs=2)
                    opsum = opsums[(j, h)]
                    ptt = pb.tile([128, 1536], BF16, name="ptt", tag="ptt", bufs=4)
                    if trips[t] == "diag":
                        # one wide exp covering the 896:1024 gap (cols there
                        # are stale psum, never read by PV)
                        nc.scalar.activation(ptt[:, 0:1408], st_tiles[m][:, 0:1408],
                                             AF.Exp)
                        for dd in range(4):
                            nc.gpsimd.tensor_mul(
                                ptt[:, DOFF[dd]:DOFF[dd] + 128],
                                ptt[:, DOFF[dd]:DOFF[dd] + 128],
                                cm_sb[:, dd, 128 * dd:128 * dd + 128])
                    else:
                        width = 512 * len(trips[t])
                        nc.scalar.activation(ptt[:, 0:width], st_tiles[m][:, 0:width],
                                             AF.Exp)
                    if m + 2 < M:
                        emit_scores(m + 2)
                    if trips[t] == "diag":
                        for dd in range(4):
                            nc.tensor.matmul(
                                opsum[0:65, 128 * dd:512],
                                v_sb[:, 4 * j + dd, h, :],
                                ptt[:, DOFF[dd]:DOFF[dd] + DW[dd]],
                                start=(j == 0 and dd == 0), stop=(dd == 3))
                    else:
                        for n, i in enumerate(trips[t]):
                            nc.tensor.matmul(
                                opsum[0:65, :],
                                v_sb[:, i, h, :],
                                ptt[:, 512 * n:512 * n + 512],
                                start=(t == 0 and n == 0), stop=False)
                    if t == ntrip - 1:
                        emit_norm(h, j, opsum)
                        del opsums[(j, h)]
                emit_phase_c(JORD[-1], list(range(8)), alt=True)

    nc.compile()
    return nc


def _run(inputs, trace=False, debug=False, tmpdir=None):
    global _NC
    if _NC is None:
        _NC = _build(debug=debug)
    q = np.asarray(inputs["q"], dtype=np.float32)
    k = np.asarray(inputs["k"], dtype=np.float32)
    v = np.asarray(inputs["v"], dtype=np.float32)
    mask = np.asarray(inputs["mask"])
    w_query = np.asarray(inputs["w_query"], dtype=np.float32)
    w_key = np.asarray(inputs["w_key"], dtype=np.float32)
    w_value = np.asarray(inputs["w_value"], dtype=np.float32)
    w_0 = np.asarray(inputs["w_0"], dtype=np.float32)

    cmask = np.stack([
        np.ascontiguousarray(mask[0, 0, 0:512, 128 * i:128 * i + 128].T)
        for i in range(4)
    ]).astype(ml_dtypes.bfloat16)
    xq_b = [np.ascontiguousarray(q[b].T).astype(np.float16) for b in range(B)]
    xk_b = [np.ascontiguousarray(k[b].T).astype(np.float16) for b in range(B)]
    xv_b = [np.ascontiguousarray(v[b].T).astype(ml_dtypes.bfloat16) for b in range(B)]

    in_maps = []
    for c in range(8):
        b, g = c // 4, c % 4
        sl = slice(256 * g, 256 * g + 256)
        in_maps.append({
            "xq": xq_b[b], "xk": xk_b[b], "xv": xv_b[b],
            "wq": np.ascontiguousarray(w_query[sl, :].T).astype(np.float16),
            "wk": np.ascontiguousarray(w_key[sl, :].T).astype(np.float16),
            "wv": np.ascontiguousarray(w_value[sl, :].T).astype(ml_dtypes.bfloat16),
            "w0": np.ascontiguousarray(w_0[:, sl].T).astype(np.float16),
            "cm": cmask,
        })

    res = run_bass_kernel_spmd(_NC, in_maps, core_ids=list(range(8)), trace=trace,
                               tmpdir=tmpdir)
    y = np.empty((B, S, D), dtype=np.float32)
    for b in range(B):
        acc = res.results[4 * b]["yt"].astype(np.float32)
        for g in range(1, 4):
            acc += res.results[4 * b + g]["yt"].astype(np.float32)
        y[b] = acc.T
    if debug:
        return y, getattr(res, "exec_time_ns", None), res
    return y, getattr(res, "exec_time_ns", None)


def kernel(**inputs):
    return _run(inputs, trace=False)[0]
